# revision 3
# baseline (speedup 1.0000x reference)
"""Self-contained Trainium2 Bass kernel for the attention-like module:

    Q = x @ Wq.T + bq;  K = x @ Wk.T + bk;  V = x @ Wv.T + bv
    S = (Q.T @ K) / sqrt(dk);  A = softmax(S, axis=0);  out = V @ A

Sharding: data-parallel over the N=8192 rows across 8 NeuronCores; each core
computes its partial S_i = Q_i.T @ K_i, an fp16 AllReduce sums them (per
512-column half so the second half's compute hides the first's collective),
each core then applies the softmax and computes its row-shard of the output.

Schedule notes (v2):
  - PE program order: Q, K h0, S h0 (-> AR#1), K h1, S h1 (-> AR#2),
    V cols 0:512, den h0, V cols 512:1024, out h0, den h1, out h1.
    The denominator matmuls are placed so the PE stream never waits on a
    collective: by the time the PE reaches den h_i, AR#i has long finished.
  - Two HWDGE rings: bulk loads + S bounce-outs + output writes on the sync
    ring; small-but-urgent bias loads and the post-collective softmax-input
    loads on the scalar ring (so they are not stuck behind 8MB of weights).
  - AllReduce payload is fp16 (f32 costs ~44us per half at ~50GB/s, fp16
    halves that; rel-err impact ~1e-3).
  - Output is evicted and DMA'd as bf16, upcast to f32 on host.
"""

import numpy as np
import ml_dtypes

N, D, NCORES, P, F = 8192, 1024, 8, 128, 512
NPC = N // NCORES   # rows per core
KC = D // P         # contraction chunks (8)
NCH = NPC // P      # row chunks per core (8)
JC = D // F         # 512-wide free chunks (2)
NORM = 1.0 / float(np.sqrt(D))
SPLITS = [(0, 512), (512, 512)]  # (col_start, width) per AllReduce block


def jtiles(c0, w):
    """split a column block into PSUM-bank-sized (<=512) tiles"""
    out = []
    js = c0
    while js < c0 + w:
        jw = min(F, c0 + w - js)
        out.append((js, jw))
        js += jw
    return out


_cache = {}


def _build_nc():
    import concourse.mybir as mybir
    import concourse.tile as tile
    from concourse import bacc

    f32 = mybir.dt.float32
    f16 = mybir.dt.float16
    bf16 = mybir.dt.bfloat16
    add = mybir.AluOpType.add
    mult = mybir.AluOpType.mult

    nc = bacc.Bacc("TRN2", target_bir_lowering=False, debug=False,
                   num_devices=NCORES)

    # x and Wq are interleaved host-side into one buffer, laid out per row as
    # [x 0:512 | wq 0:512 | x 512:1024 | wq 512:1024], so each contraction
    # chunk (the lhsT+rhs pair the PE needs next) arrives as ONE dma
    XQ = nc.dram_tensor("XQ", [D, NPC + D], bf16, kind="ExternalInput").ap()
    WkT = nc.dram_tensor("WkT", [D, D], bf16, kind="ExternalInput").ap()
    WvT = nc.dram_tensor("WvT", [D, D], bf16, kind="ExternalInput").ap()
    bqr = nc.dram_tensor("bqr", [P, D], f32, kind="ExternalInput").ap()
    bkr = nc.dram_tensor("bkr", [P, D], f32, kind="ExternalInput").ap()
    bvc = nc.dram_tensor("bvc", [P, KC], f32, kind="ExternalInput").ap()
    out = nc.dram_tensor("out", [NPC, D], bf16, kind="ExternalOutput").ap()

    with tile.TileContext(nc) as tc:
        with tc.tile_pool(name="persist", bufs=1) as pp, \
             tc.tile_pool(name="stage", bufs=6) as sp, \
             tc.tile_pool(name="sin", bufs=2) as sip, \
             tc.tile_pool(name="psA", bufs=7, space="PSUM") as psA, \
             tc.tile_pool(name="psB", bufs=1, space="PSUM") as psB, \
             tc.tile_pool(name="dram", bufs=1, space="DRAM") as dp:

            # ---- resident inputs; bulk loads on the sync ring in the order
            # the PE consumes them, biases on the scalar ring so Q-tile
            # evictions are never blocked behind the weight stream ----
            W2 = NPC + D
            xq = pp.tile([P, KC, W2], bf16, name="xq")
            xqr = XQ.rearrange("(kc p) n -> p kc n", p=P)
            wk = pp.tile([P, KC, D], bf16, name="wk")
            wkr = WkT.rearrange("(kc p) j -> p kc j", p=P)
            wv = pp.tile([P, KC, D], bf16, name="wv")
            wvr = WvT.rearrange("(kc p) j -> p kc j", p=P)
            for kc in range(KC):
                nc.sync.dma_start(xq[:, kc, 0:W2 // 2], xqr[:, kc, 0:W2 // 2])
                nc.sync.dma_start(xq[:, kc, W2 // 2:], xqr[:, kc, W2 // 2:])
            for kc in range(KC):
                nc.sync.dma_start(wk[:, kc], wkr[:, kc])
            for kc in range(KC):
                nc.sync.dma_start(wv[:, kc], wvr[:, kc])

            def xts(kc, a, b):
                # x columns [a:b) within the interleaved xq layout
                off = 0 if b <= F else F
                return xq[:, kc, a + off:b + off]

            def wqs(kc, a, b):
                # wq columns [a:b) within the interleaved xq layout
                off = F if b <= F else 2 * F
                return xq[:, kc, a + off:b + off]
            bq_sb = pp.tile([P, D], f32, name="bq_sb")
            nc.scalar.dma_start(bq_sb[:], bqr[:])
            bk_sb = pp.tile([P, D], f32, name="bk_sb")
            nc.scalar.dma_start(bk_sb[:], bkr[:])
            bv_sb = pp.tile([P, KC], f32, name="bv_sb")
            nc.scalar.dma_start(bv_sb[:], bvc[:])
            ones_b = pp.tile([P, 1], bf16, name="ones_b")
            nc.any.memset(ones_b[:], 1.0)
            ones_f = pp.tile([1, P], f32, name="ones_f")
            nc.any.memset(ones_f[:], 1.0)

            # ---- Q projection (full) ----
            # First 7 tiles run kc-major across 7 concurrent PSUM banks so the
            # PE consumes each (xt,wq) chunk pair as soon as it lands instead
            # of serializing tile-major behind the full input stream.
            q_sb = pp.tile([P, NCH, D], bf16, name="q_sb")
            k_sb = pp.tile([P, NCH, D], bf16, name="k_sb")
            g0 = [(0, 0), (1, 0), (2, 0), (3, 0), (0, 1), (1, 1), (2, 1)]
            ps_g0 = [psA.tile([P, F], f32, tag="psA", name="ps_q0")
                     for _ in g0]
            for kc in range(KC):
                for t, (nch, jc) in enumerate(g0):
                    nc.tensor.matmul(
                        ps_g0[t][:],
                        xts(kc, nch * P, (nch + 1) * P),
                        wqs(kc, jc * F, (jc + 1) * F),
                        start=(kc == 0), stop=(kc == KC - 1))
            for t, (nch, jc) in enumerate(g0):
                nc.vector.tensor_tensor(
                    q_sb[:, nch, jc * F:(jc + 1) * F],
                    ps_g0[t][:], bq_sb[:, jc * F:(jc + 1) * F], add)
            for nch in range(NCH):
                for jc in range(JC):
                    if (nch, jc) in g0:
                        continue
                    ps = psA.tile([P, F], f32, tag="psA", name="ps_q")
                    for kc in range(KC):
                        nc.tensor.matmul(
                            ps[:],
                            xts(kc, nch * P, (nch + 1) * P),
                            wqs(kc, jc * F, (jc + 1) * F),
                            start=(kc == 0), stop=(kc == KC - 1))
                    nc.vector.tensor_tensor(
                        q_sb[:, nch, jc * F:(jc + 1) * F],
                        ps[:], bq_sb[:, jc * F:(jc + 1) * F], add)

            # tiny dummy AllReduce issued up front (hidden under Q/K compute)
            # to absorb the collective path's cold-start cost: the first real
            # AllReduce otherwise runs ~10us slower than the second
            warm_in = dp.tile([P, 16], f16, name="warm_in")
            warm_out = dp.tile([P, 16], f16, name="warm_out",
                               addr_space="Shared")
            warm_sb = pp.tile([P, 16], f16, name="warm_sb")
            nc.any.memset(warm_sb[:], 0.0)
            nc.sync.dma_start(warm_in[:], warm_sb[:])
            nc.gpsimd.collective_compute(
                "AllReduce", add,
                replica_groups=[list(range(NCORES))],
                ins=[warm_in.opt()], outs=[warm_out.opt()])

            # scores split into two column blocks (384 / 640); each block is
            # projected (K), contracted (S), and all-reduced independently.
            # The FIRST block is the small one so AR#1 triggers ~12us earlier;
            # the chained AR#2 then also completes earlier, and the larger
            # final out block provides more tail work to hide it.
            s_bounce = [dp.tile([D, w], f16, name=f"s_bounce{h}")
                        for h, (c0, w) in enumerate(SPLITS)]
            s_red = [dp.tile([D, w], f16, name=f"s_red{h}",
                             addr_space="Shared")
                     for h, (c0, w) in enumerate(SPLITS)]
            e_sb = pp.tile([P, KC, D], bf16, name="e_sb")
            for h, (c0, w) in enumerate(SPLITS):
                # K columns for this block
                for nch in range(NCH):
                    for js, jw in jtiles(c0, w):
                        ps = psA.tile([P, jw], f32, tag="psA", name="ps_k")
                        for kc in range(KC):
                            nc.tensor.matmul(
                                ps[:],
                                xts(kc, nch * P, (nch + 1) * P),
                                wk[:, kc, js:js + jw],
                                start=(kc == 0), stop=(kc == KC - 1))
                        nc.vector.tensor_tensor(
                            k_sb[:, nch, js:js + jw],
                            ps[:], bk_sb[:, js:js + jw], add)
                # partial scores for this block: [all qi, these j-cols]
                for qch in range(KC):
                    for js, jw in jtiles(c0, w):
                        st = sp.tile([P, jw], f16, tag="sstage", name="st")
                        ps = psA.tile([P, jw], f32, tag="psA", name="ps_s")
                        for nch in range(NCH):
                            nc.tensor.matmul(
                                ps[:],
                                q_sb[:, nch, qch * P:(qch + 1) * P],
                                k_sb[:, nch, js:js + jw],
                                start=(nch == 0), stop=(nch == NCH - 1))
                        nc.scalar.activation(
                            st[:], ps[:],
                            mybir.ActivationFunctionType.Copy)
                        nc.sync.dma_start(
                            s_bounce[h][qch * P:(qch + 1) * P,
                                        js - c0:js - c0 + jw], st[:])
                nc.gpsimd.collective_compute(
                    "AllReduce", add,
                    replica_groups=[list(range(NCORES))],
                    ins=[s_bounce[h].opt()], outs=[s_red[h].opt()])

            # softmax-input loads + exp on the scalar ring/engine, in ~256KB
            # pieces (small loads serialize ~2us receipts, one big load
            # delays the first exp). Emitted AFTER both AR triggers: exp h0
            # blocks the scalar queue on AR#1, so nothing that gates AR#2
            # may be queued behind it.
            for h, (c0, w) in enumerate(SPLITS):
                s_t = sip.tile([P, KC, w], f16, tag="sin", name="s_t")
                s_rr = s_red[h].rearrange("(ich p) f -> p ich f", p=P)
                for q4 in range(4):
                    nc.scalar.dma_start(
                        s_t[:, 2 * q4:2 * q4 + 2], s_rr[:, 2 * q4:2 * q4 + 2])
                for ich in range(KC):
                    nc.scalar.activation(
                        e_sb[:, ich, c0:c0 + w], s_t[:, ich],
                        mybir.ActivationFunctionType.Exp, scale=NORM)

            # ---- V.T projection (independent of both AllReduces), split in
            # two column halves with the h0 softmax denominator in between so
            # the PE reaches it only after AR#1 + exp h0 are long done ----
            vt_sb = pp.tile([P, KC, NPC], bf16, name="vt_sb")
            den_sb = pp.tile([1, D], f32, name="den_sb")
            rden_sb = pp.tile([P, D], f32, name="rden_sb")

            def vproj_half(jc2):
                for ich in range(KC):
                    ps = psA.tile([P, F], f32, tag="psA", name="ps_v")
                    for kc in range(KC):
                        nc.tensor.matmul(
                            ps[:],
                            wv[:, kc, ich * P:(ich + 1) * P],
                            xts(kc, jc2 * F, (jc2 + 1) * F),
                            start=(kc == 0), stop=(kc == KC - 1))
                    nc.vector.tensor_scalar(
                        vt_sb[:, ich, jc2 * F:(jc2 + 1) * F],
                        ps[:], bv_sb[:, ich:ich + 1], None, add)

            def out_half(h):
                # denominator matmuls are interleaved with the first out
                # tile's accumulation: both consume exp chunk ich as the
                # scalar engine produces it, so nothing serializes behind the
                # full exp stream after the AllReduce lands
                c0, w = SPLITS[h]
                for js, jw in jtiles(c0, w):
                    psd = psB.tile([1, jw], f32, tag="psB", name="psd")
                    for nch in range(NCH):
                        ot = sp.tile([P, jw], bf16, tag="ostage", name="ot")
                        ps = psA.tile([P, jw], f32, tag="psA", name="ps_o")
                        for ich in range(KC):
                            nc.tensor.matmul(
                                ps[:],
                                vt_sb[:, ich, nch * P:(nch + 1) * P],
                                e_sb[:, ich, js:js + jw],
                                start=(ich == 0), stop=(ich == KC - 1))
                            if nch == 0:
                                nc.tensor.matmul(
                                    psd[:], ones_b[:, 0:1],
                                    e_sb[:, ich, js:js + jw],
                                    start=(ich == 0), stop=(ich == KC - 1))
                        if nch == 0:
                            nc.vector.tensor_copy(
                                den_sb[:, js:js + jw], psd[:])
                            psr = psB.tile([P, jw], f32, tag="psB", name="psr")
                            nc.tensor.matmul(
                                psr[:], ones_f[:, 0:P],
                                den_sb[:, js:js + jw],
                                start=True, stop=True)
                            nc.vector.reciprocal(
                                rden_sb[:, js:js + jw], psr[:])
                        nc.vector.tensor_tensor(
                            ot[:], ps[:], rden_sb[:, js:js + jw], mult)
                        nc.sync.dma_start(
                            out[nch * P:(nch + 1) * P, js:js + jw], ot[:])

            vproj_half(0)
            vproj_half(1)
            out_half(0)
            out_half(1)

    nc.compile()
    return nc


def _prep_inputs(x, Wq, bq, Wk, bk, Wv, bv):
    bf16 = ml_dtypes.bfloat16
    xT_all = np.ascontiguousarray(np.asarray(x).astype(bf16).T)
    WqT = np.ascontiguousarray(np.asarray(Wq).astype(bf16).T)
    WkT = np.ascontiguousarray(np.asarray(Wk).astype(bf16).T)
    WvT = np.ascontiguousarray(np.asarray(Wv).astype(bf16).T)
    bqr = np.ascontiguousarray(
        np.broadcast_to(np.asarray(bq, np.float32), (P, D)))
    bkr = np.ascontiguousarray(
        np.broadcast_to(np.asarray(bk, np.float32), (P, D)))
    bvc = np.ascontiguousarray(
        np.asarray(bv, np.float32).reshape(KC, P).T)
    in_maps = []
    for c in range(NCORES):
        shard = xT_all[:, c * NPC:(c + 1) * NPC]
        xq = np.ascontiguousarray(np.concatenate(
            [shard[:, 0:F], WqT[:, 0:F], shard[:, F:], WqT[:, F:]], axis=1))
        in_maps.append({
            "XQ": xq, "WkT": WkT, "WvT": WvT,
            "bqr": bqr, "bkr": bkr, "bvc": bvc,
        })
    return in_maps


def _ensure_axon_hooks_stub():
    # bass_utils imports antenv.axon_hooks when tracing is requested (also
    # via the BASS_TRACE env var); this image ships antenv without that
    # submodule, so install a no-op stub to degrade gracefully.
    import sys
    import types
    try:
        import antenv.axon_hooks  # noqa: F401
        return
    except ImportError:
        pass
    mod = types.ModuleType("antenv.axon_hooks")
    mod._hook = None
    mod.set_axon_ntff_profile_hook = lambda h: setattr(mod, "_hook", h)
    mod.get_axon_ntff_profile_hook = lambda: mod._hook
    sys.modules["antenv.axon_hooks"] = mod
    try:
        import antenv
        antenv.axon_hooks = mod
    except ImportError:
        pass


def kernel(x, Wq, bq, Wk, bk, Wv, bv, _trace=False):
    from concourse import bass_utils

    _ensure_axon_hooks_stub()

    if "nc" not in _cache:
        _cache["nc"] = _build_nc()
    nc = _cache["nc"]

    in_maps = _prep_inputs(x, Wq, bq, Wk, bk, Wv, bv)
    res = bass_utils.run_bass_kernel_spmd(
        nc, in_maps, core_ids=list(range(NCORES)), trace=_trace)
    _cache["last_result"] = res
    return np.concatenate(
        [np.asarray(res.results[c]["out"], dtype=np.float32)
         for c in range(NCORES)], axis=0)


# revision 4
# speedup vs baseline: 1.0209x; 1.0209x over previous
"""Self-contained Trainium2 Bass kernel for the attention-like module:

    Q = x @ Wq.T + bq;  K = x @ Wk.T + bk;  V = x @ Wv.T + bv
    S = (Q.T @ K) / sqrt(dk);  A = softmax(S, axis=0);  out = V @ A

Sharding: data-parallel over the N=8192 rows across 8 NeuronCores; each core
computes its partial S_i = Q_i.T @ K_i, an fp16 AllReduce sums them (per
512-column half so the second half's compute hides the first's collective),
each core then applies the softmax and computes its row-shard of the output.

Schedule notes (v2):
  - PE program order: Q, K h0, S h0 (-> AR#1), K h1, S h1 (-> AR#2),
    V cols 0:512, den h0, V cols 512:1024, out h0, den h1, out h1.
    The denominator matmuls are placed so the PE stream never waits on a
    collective: by the time the PE reaches den h_i, AR#i has long finished.
  - Two HWDGE rings: bulk loads + S bounce-outs + output writes on the sync
    ring; small-but-urgent bias loads and the post-collective softmax-input
    loads on the scalar ring (so they are not stuck behind 8MB of weights).
  - AllReduce payload is fp16 (f32 costs ~44us per half at ~50GB/s, fp16
    halves that; rel-err impact ~1e-3).
  - Output is evicted and DMA'd as bf16, upcast to f32 on host.
"""

import numpy as np
import ml_dtypes

N, D, NCORES, P, F = 8192, 1024, 8, 128, 512
NPC = N // NCORES   # rows per core
KC = D // P         # contraction chunks (8)
NCH = NPC // P      # row chunks per core (8)
JC = D // F         # 512-wide free chunks (2)
NORM = 1.0 / float(np.sqrt(D))
SPLITS = [(0, 512), (512, 512)]  # (col_start, width) per AllReduce block


def jtiles(c0, w):
    """split a column block into PSUM-bank-sized (<=512) tiles"""
    out = []
    js = c0
    while js < c0 + w:
        jw = min(F, c0 + w - js)
        out.append((js, jw))
        js += jw
    return out


_cache = {}


def _build_nc():
    import concourse.mybir as mybir
    import concourse.tile as tile
    from concourse import bacc

    f32 = mybir.dt.float32
    f16 = mybir.dt.float16
    bf16 = mybir.dt.bfloat16
    add = mybir.AluOpType.add
    mult = mybir.AluOpType.mult

    nc = bacc.Bacc("TRN2", target_bir_lowering=False, debug=False,
                   num_devices=NCORES)

    # x and Wq are interleaved host-side into one buffer, laid out per row as
    # [x 0:512 | wq 0:512 | x 512:1024 | wq 512:1024], so each contraction
    # chunk (the lhsT+rhs pair the PE needs next) arrives as ONE dma
    XQ = nc.dram_tensor("XQ", [D, NPC + D], bf16, kind="ExternalInput").ap()
    WkT = nc.dram_tensor("WkT", [D, D], bf16, kind="ExternalInput").ap()
    WvT = nc.dram_tensor("WvT", [D, D], bf16, kind="ExternalInput").ap()
    bqr = nc.dram_tensor("bqr", [P, D], bf16, kind="ExternalInput").ap()
    bkr = nc.dram_tensor("bkr", [P, D], bf16, kind="ExternalInput").ap()
    bvc = nc.dram_tensor("bvc", [P, KC], f32, kind="ExternalInput").ap()
    out = nc.dram_tensor("out", [NPC, D], bf16, kind="ExternalOutput").ap()

    with tile.TileContext(nc) as tc:
        with tc.tile_pool(name="persist", bufs=1) as pp, \
             tc.tile_pool(name="stage", bufs=6) as sp, \
             tc.tile_pool(name="sin", bufs=2) as sip, \
             tc.tile_pool(name="psA", bufs=7, space="PSUM") as psA, \
             tc.tile_pool(name="psB", bufs=1, space="PSUM") as psB, \
             tc.tile_pool(name="dram", bufs=1, space="DRAM") as dp:

            # ---- resident inputs; bulk loads on the sync ring in the order
            # the PE consumes them, biases on the scalar ring so Q-tile
            # evictions are never blocked behind the weight stream ----
            W2 = NPC + D
            xq = pp.tile([P, KC, W2], bf16, name="xq")
            xqr = XQ.rearrange("(kc p) n -> p kc n", p=P)
            wk = pp.tile([P, KC, D], bf16, name="wk")
            wkr = WkT.rearrange("(kc p) j -> p kc j", p=P)
            wv = pp.tile([P, KC, D], bf16, name="wv")
            wvr = WvT.rearrange("(kc p) j -> p kc j", p=P)
            # each chunk's two halves stream on the two independent HWDGE
            # rings (sync + scalar) in parallel: one ring alone sustains only
            # ~190GB/s with 2KB descriptors, which starves the 7-bank-limited
            # Q-projection start
            for kc in range(KC):
                nc.sync.dma_start(xq[:, kc, 0:W2 // 2], xqr[:, kc, 0:W2 // 2])
                nc.scalar.dma_start(xq[:, kc, W2 // 2:], xqr[:, kc, W2 // 2:])
            for kc in range(KC):
                nc.sync.dma_start(wk[:, kc], wkr[:, kc])
            for kc in range(KC):
                nc.sync.dma_start(wv[:, kc], wvr[:, kc])

            def xts(kc, a, b):
                # x columns [a:b) within the interleaved xq layout
                off = 0 if b <= F else F
                return xq[:, kc, a + off:b + off]

            def wqs(kc, a, b):
                # wq columns [a:b) within the interleaved xq layout
                off = F if b <= F else 2 * F
                return xq[:, kc, a + off:b + off]
            bq_sb = pp.tile([P, D], bf16, name="bq_sb")
            nc.scalar.dma_start(bq_sb[:], bqr[:])
            bk_sb = pp.tile([P, D], bf16, name="bk_sb")
            nc.scalar.dma_start(bk_sb[:], bkr[:])
            bv_sb = pp.tile([P, KC], f32, name="bv_sb")
            nc.scalar.dma_start(bv_sb[:], bvc[:])
            ones_b = pp.tile([P, 1], bf16, name="ones_b")
            nc.any.memset(ones_b[:], 1.0)
            ones_f = pp.tile([1, P], f32, name="ones_f")
            nc.any.memset(ones_f[:], 1.0)

            # ---- Q projection (full) ----
            # First 7 tiles run kc-major across 7 concurrent PSUM banks so the
            # PE consumes each (xt,wq) chunk pair as soon as it lands instead
            # of serializing tile-major behind the full input stream.
            q_sb = pp.tile([P, NCH, D], bf16, name="q_sb")
            k_sb = pp.tile([P, NCH, D], bf16, name="k_sb")
            g0 = [(0, 0), (1, 0), (2, 0), (3, 0), (0, 1), (1, 1), (2, 1)]
            ps_g0 = [psA.tile([P, F], f32, tag="psA", name="ps_q0")
                     for _ in g0]
            for kc in range(KC):
                for t, (nch, jc) in enumerate(g0):
                    nc.tensor.matmul(
                        ps_g0[t][:],
                        xts(kc, nch * P, (nch + 1) * P),
                        wqs(kc, jc * F, (jc + 1) * F),
                        start=(kc == 0), stop=(kc == KC - 1))
            for t, (nch, jc) in enumerate(g0):
                nc.vector.tensor_tensor(
                    q_sb[:, nch, jc * F:(jc + 1) * F],
                    ps_g0[t][:], bq_sb[:, jc * F:(jc + 1) * F], add)
            for nch in range(NCH):
                for jc in range(JC):
                    if (nch, jc) in g0:
                        continue
                    ps = psA.tile([P, F], f32, tag="psA", name="ps_q")
                    for kc in range(KC):
                        nc.tensor.matmul(
                            ps[:],
                            xts(kc, nch * P, (nch + 1) * P),
                            wqs(kc, jc * F, (jc + 1) * F),
                            start=(kc == 0), stop=(kc == KC - 1))
                    nc.vector.tensor_tensor(
                        q_sb[:, nch, jc * F:(jc + 1) * F],
                        ps[:], bq_sb[:, jc * F:(jc + 1) * F], add)

            # tiny dummy AllReduce issued up front (hidden under Q/K compute)
            # to absorb the collective path's cold-start cost: the first real
            # AllReduce otherwise runs ~10us slower than the second
            warm_in = dp.tile([P, 16], f16, name="warm_in")
            warm_out = dp.tile([P, 16], f16, name="warm_out",
                               addr_space="Shared")
            warm_sb = pp.tile([P, 16], f16, name="warm_sb")
            nc.any.memset(warm_sb[:], 0.0)
            nc.sync.dma_start(warm_in[:], warm_sb[:])
            nc.gpsimd.collective_compute(
                "AllReduce", add,
                replica_groups=[list(range(NCORES))],
                ins=[warm_in.opt()], outs=[warm_out.opt()])

            # scores split into two column blocks (384 / 640); each block is
            # projected (K), contracted (S), and all-reduced independently.
            # The FIRST block is the small one so AR#1 triggers ~12us earlier;
            # the chained AR#2 then also completes earlier, and the larger
            # final out block provides more tail work to hide it.
            s_bounce = [dp.tile([D, w], f16, name=f"s_bounce{h}")
                        for h, (c0, w) in enumerate(SPLITS)]
            s_red = [dp.tile([D, w], f16, name=f"s_red{h}",
                             addr_space="Shared")
                     for h, (c0, w) in enumerate(SPLITS)]
            e_sb = pp.tile([P, KC, D], bf16, name="e_sb")
            for h, (c0, w) in enumerate(SPLITS):
                # K columns for this block
                for nch in range(NCH):
                    for js, jw in jtiles(c0, w):
                        ps = psA.tile([P, jw], f32, tag="psA", name="ps_k")
                        for kc in range(KC):
                            nc.tensor.matmul(
                                ps[:],
                                xts(kc, nch * P, (nch + 1) * P),
                                wk[:, kc, js:js + jw],
                                start=(kc == 0), stop=(kc == KC - 1))
                        nc.vector.tensor_tensor(
                            k_sb[:, nch, js:js + jw],
                            ps[:], bk_sb[:, js:js + jw], add)
                # partial scores for this block: [all qi, these j-cols]
                for qch in range(KC):
                    for js, jw in jtiles(c0, w):
                        st = sp.tile([P, jw], f16, tag="sstage", name="st")
                        ps = psA.tile([P, jw], f32, tag="psA", name="ps_s")
                        for nch in range(NCH):
                            nc.tensor.matmul(
                                ps[:],
                                q_sb[:, nch, qch * P:(qch + 1) * P],
                                k_sb[:, nch, js:js + jw],
                                start=(nch == 0), stop=(nch == NCH - 1))
                        nc.scalar.activation(
                            st[:], ps[:],
                            mybir.ActivationFunctionType.Copy)
                        nc.sync.dma_start(
                            s_bounce[h][qch * P:(qch + 1) * P,
                                        js - c0:js - c0 + jw], st[:])
                nc.gpsimd.collective_compute(
                    "AllReduce", add,
                    replica_groups=[list(range(NCORES))],
                    ins=[s_bounce[h].opt()], outs=[s_red[h].opt()])

            # softmax-input loads + exp on the scalar ring/engine, in ~256KB
            # pieces (small loads serialize ~2us receipts, one big load
            # delays the first exp). Emitted AFTER both AR triggers: exp h0
            # blocks the scalar queue on AR#1, so nothing that gates AR#2
            # may be queued behind it.
            for h, (c0, w) in enumerate(SPLITS):
                s_t = sip.tile([P, KC, w], f16, tag="sin", name="s_t")
                s_rr = s_red[h].rearrange("(ich p) f -> p ich f", p=P)
                for q4 in range(4):
                    nc.scalar.dma_start(
                        s_t[:, 2 * q4:2 * q4 + 2], s_rr[:, 2 * q4:2 * q4 + 2])
                for ich in range(KC):
                    nc.scalar.activation(
                        e_sb[:, ich, c0:c0 + w], s_t[:, ich],
                        mybir.ActivationFunctionType.Exp, scale=NORM)

            # ---- V.T projection (independent of both AllReduces), split in
            # two column halves with the h0 softmax denominator in between so
            # the PE reaches it only after AR#1 + exp h0 are long done ----
            vt_sb = pp.tile([P, KC, NPC], bf16, name="vt_sb")
            den_sb = pp.tile([1, D], f32, name="den_sb")
            rden_sb = pp.tile([P, D], f32, name="rden_sb")

            def vproj_half(jc2):
                for ich in range(KC):
                    ps = psA.tile([P, F], f32, tag="psA", name="ps_v")
                    for kc in range(KC):
                        nc.tensor.matmul(
                            ps[:],
                            wv[:, kc, ich * P:(ich + 1) * P],
                            xts(kc, jc2 * F, (jc2 + 1) * F),
                            start=(kc == 0), stop=(kc == KC - 1))
                    nc.vector.tensor_scalar(
                        vt_sb[:, ich, jc2 * F:(jc2 + 1) * F],
                        ps[:], bv_sb[:, ich:ich + 1], None, add)

            def out_half(h):
                # denominator matmuls are interleaved with the first out
                # tile's accumulation: both consume exp chunk ich as the
                # scalar engine produces it, so nothing serializes behind the
                # full exp stream after the AllReduce lands
                c0, w = SPLITS[h]
                for js, jw in jtiles(c0, w):
                    psd = psB.tile([1, jw], f32, tag="psB", name="psd")
                    for nch in range(NCH):
                        ot = sp.tile([P, jw], bf16, tag="ostage", name="ot")
                        ps = psA.tile([P, jw], f32, tag="psA", name="ps_o")
                        for ich in range(KC):
                            nc.tensor.matmul(
                                ps[:],
                                vt_sb[:, ich, nch * P:(nch + 1) * P],
                                e_sb[:, ich, js:js + jw],
                                start=(ich == 0), stop=(ich == KC - 1))
                            if nch == 0:
                                nc.tensor.matmul(
                                    psd[:], ones_b[:, 0:1],
                                    e_sb[:, ich, js:js + jw],
                                    start=(ich == 0), stop=(ich == KC - 1))
                        if nch == 0:
                            nc.vector.tensor_copy(
                                den_sb[:, js:js + jw], psd[:])
                            psr = psB.tile([P, jw], f32, tag="psB", name="psr")
                            nc.tensor.matmul(
                                psr[:], ones_f[:, 0:P],
                                den_sb[:, js:js + jw],
                                start=True, stop=True)
                            nc.vector.reciprocal(
                                rden_sb[:, js:js + jw], psr[:])
                        nc.vector.tensor_tensor(
                            ot[:], ps[:], rden_sb[:, js:js + jw], mult)
                        nc.sync.dma_start(
                            out[nch * P:(nch + 1) * P, js:js + jw], ot[:])

            vproj_half(0)
            vproj_half(1)
            out_half(0)
            out_half(1)

    nc.compile()
    return nc


def _prep_inputs(x, Wq, bq, Wk, bk, Wv, bv):
    bf16 = ml_dtypes.bfloat16
    xT_all = np.ascontiguousarray(np.asarray(x).astype(bf16).T)
    WqT = np.ascontiguousarray(np.asarray(Wq).astype(bf16).T)
    WkT = np.ascontiguousarray(np.asarray(Wk).astype(bf16).T)
    WvT = np.ascontiguousarray(np.asarray(Wv).astype(bf16).T)
    bqr = np.ascontiguousarray(
        np.broadcast_to(np.asarray(bq, np.float32).astype(bf16), (P, D)))
    bkr = np.ascontiguousarray(
        np.broadcast_to(np.asarray(bk, np.float32).astype(bf16), (P, D)))
    bvc = np.ascontiguousarray(
        np.asarray(bv, np.float32).reshape(KC, P).T)
    in_maps = []
    for c in range(NCORES):
        shard = xT_all[:, c * NPC:(c + 1) * NPC]
        xq = np.ascontiguousarray(np.concatenate(
            [shard[:, 0:F], WqT[:, 0:F], shard[:, F:], WqT[:, F:]], axis=1))
        in_maps.append({
            "XQ": xq, "WkT": WkT, "WvT": WvT,
            "bqr": bqr, "bkr": bkr, "bvc": bvc,
        })
    return in_maps


def _ensure_axon_hooks_stub():
    # bass_utils imports antenv.axon_hooks when tracing is requested (also
    # via the BASS_TRACE env var); this image ships antenv without that
    # submodule, so install a no-op stub to degrade gracefully.
    import sys
    import types
    try:
        import antenv.axon_hooks  # noqa: F401
        return
    except ImportError:
        pass
    mod = types.ModuleType("antenv.axon_hooks")
    mod._hook = None
    mod.set_axon_ntff_profile_hook = lambda h: setattr(mod, "_hook", h)
    mod.get_axon_ntff_profile_hook = lambda: mod._hook
    sys.modules["antenv.axon_hooks"] = mod
    try:
        import antenv
        antenv.axon_hooks = mod
    except ImportError:
        pass


def kernel(x, Wq, bq, Wk, bk, Wv, bv, _trace=False):
    from concourse import bass_utils

    _ensure_axon_hooks_stub()

    if "nc" not in _cache:
        _cache["nc"] = _build_nc()
    nc = _cache["nc"]

    in_maps = _prep_inputs(x, Wq, bq, Wk, bk, Wv, bv)
    res = bass_utils.run_bass_kernel_spmd(
        nc, in_maps, core_ids=list(range(NCORES)), trace=_trace)
    _cache["last_result"] = res
    return np.concatenate(
        [np.asarray(res.results[c]["out"], dtype=np.float32)
         for c in range(NCORES)], axis=0)


# revision 5
# speedup vs baseline: 1.0279x; 1.0069x over previous
"""Self-contained Trainium2 Bass kernel for the attention-like module:

    Q = x @ Wq.T + bq;  K = x @ Wk.T + bk;  V = x @ Wv.T + bv
    S = (Q.T @ K) / sqrt(dk);  A = softmax(S, axis=0);  out = V @ A

Sharding: data-parallel over the N=8192 rows across 8 NeuronCores; each core
computes its partial S_i = Q_i.T @ K_i, an fp16 AllReduce sums them (per
512-column half so the second half's compute hides the first's collective),
each core then applies the softmax and computes its row-shard of the output.

Schedule notes (v2):
  - PE program order: Q, K h0, S h0 (-> AR#1), K h1, S h1 (-> AR#2),
    V cols 0:512, den h0, V cols 512:1024, out h0, den h1, out h1.
    The denominator matmuls are placed so the PE stream never waits on a
    collective: by the time the PE reaches den h_i, AR#i has long finished.
  - Two HWDGE rings: bulk loads + S bounce-outs + output writes on the sync
    ring; small-but-urgent bias loads and the post-collective softmax-input
    loads on the scalar ring (so they are not stuck behind 8MB of weights).
  - AllReduce payload is fp16 (f32 costs ~44us per half at ~50GB/s, fp16
    halves that; rel-err impact ~1e-3).
  - Output is evicted and DMA'd as bf16, upcast to f32 on host.
"""

import numpy as np
import ml_dtypes

N, D, NCORES, P, F = 8192, 1024, 8, 128, 512
NPC = N // NCORES   # rows per core
KC = D // P         # contraction chunks (8)
NCH = NPC // P      # row chunks per core (8)
JC = D // F         # 512-wide free chunks (2)
NORM = 1.0 / float(np.sqrt(D))
SPLITS = [(0, 512), (512, 512)]  # (col_start, width) per AllReduce block


def jtiles(c0, w):
    """split a column block into PSUM-bank-sized (<=512) tiles"""
    out = []
    js = c0
    while js < c0 + w:
        jw = min(F, c0 + w - js)
        out.append((js, jw))
        js += jw
    return out


_cache = {}


def _build_nc():
    import concourse.mybir as mybir
    import concourse.tile as tile
    from concourse import bacc

    f32 = mybir.dt.float32
    f16 = mybir.dt.float16
    bf16 = mybir.dt.bfloat16
    add = mybir.AluOpType.add
    mult = mybir.AluOpType.mult

    nc = bacc.Bacc("TRN2", target_bir_lowering=False, debug=False,
                   num_devices=NCORES)

    # x and Wq are interleaved host-side into one buffer, laid out per row as
    # [x 0:512 | wq 0:512 | x 512:1024 | wq 512:1024], so each contraction
    # chunk (the lhsT+rhs pair the PE needs next) arrives as ONE dma
    XQ = nc.dram_tensor("XQ", [D, NPC + D], bf16, kind="ExternalInput").ap()
    WkT = nc.dram_tensor("WkT", [D, D], bf16, kind="ExternalInput").ap()
    WvT = nc.dram_tensor("WvT", [D, D], bf16, kind="ExternalInput").ap()
    bqr = nc.dram_tensor("bqr", [P, D], bf16, kind="ExternalInput").ap()
    bkr = nc.dram_tensor("bkr", [P, D], bf16, kind="ExternalInput").ap()
    bvc = nc.dram_tensor("bvc", [P, KC], f32, kind="ExternalInput").ap()
    out = nc.dram_tensor("out", [NPC, D], bf16, kind="ExternalOutput").ap()

    with tile.TileContext(nc) as tc:
        with tc.tile_pool(name="persist", bufs=1) as pp, \
             tc.tile_pool(name="stage", bufs=6) as sp, \
             tc.tile_pool(name="sin", bufs=2) as sip, \
             tc.tile_pool(name="psA", bufs=7, space="PSUM") as psA, \
             tc.tile_pool(name="psB", bufs=1, space="PSUM") as psB, \
             tc.tile_pool(name="dram", bufs=1, space="DRAM") as dp:

            # ---- resident inputs; bulk loads on the sync ring in the order
            # the PE consumes them, biases on the scalar ring so Q-tile
            # evictions are never blocked behind the weight stream ----
            W2 = NPC + D
            xq = pp.tile([P, KC, W2], bf16, name="xq")
            xqr = XQ.rearrange("(kc p) n -> p kc n", p=P)
            wk = pp.tile([P, KC, D], bf16, name="wk")
            wkr = WkT.rearrange("(kc p) j -> p kc j", p=P)
            wv = pp.tile([P, KC, D], bf16, name="wv")
            wvr = WvT.rearrange("(kc p) j -> p kc j", p=P)
            # tiny dummy AllReduce issued FIRST: it absorbs the collective
            # path's cold-start cost (cuts the real AR#1's trigger->start
            # delay ~10us) and must complete before the real AR#1 is ready,
            # i.e. it has to ride out the variable 25-45us entry barrier
            # concurrently with the input load, not after it
            warm_in = dp.tile([P, 16], f16, name="warm_in")
            warm_out = dp.tile([P, 16], f16, name="warm_out",
                               addr_space="Shared")
            warm_sb = pp.tile([P, 16], f16, name="warm_sb")
            nc.vector.memset(warm_sb[:], 0.0)
            nc.sync.dma_start(warm_in[:], warm_sb[:])
            nc.gpsimd.collective_compute(
                "AllReduce", add,
                replica_groups=[list(range(NCORES))],
                ins=[warm_in.opt()], outs=[warm_out.opt()])

            # each chunk's two halves stream on the two independent HWDGE
            # rings (sync + scalar) in parallel: one ring alone sustains only
            # ~190GB/s with 2KB descriptors, which starves the 7-bank-limited
            # Q-projection start
            for kc in range(KC):
                nc.sync.dma_start(xq[:, kc, 0:W2 // 2], xqr[:, kc, 0:W2 // 2])
                nc.scalar.dma_start(xq[:, kc, W2 // 2:], xqr[:, kc, W2 // 2:])
            for kc in range(KC):
                nc.sync.dma_start(wk[:, kc], wkr[:, kc])
            for kc in range(KC):
                nc.sync.dma_start(wv[:, kc], wvr[:, kc])

            def xts(kc, a, b):
                # x columns [a:b) within the interleaved xq layout
                off = 0 if b <= F else F
                return xq[:, kc, a + off:b + off]

            def wqs(kc, a, b):
                # wq columns [a:b) within the interleaved xq layout
                off = F if b <= F else 2 * F
                return xq[:, kc, a + off:b + off]
            bq_sb = pp.tile([P, D], bf16, name="bq_sb")
            nc.scalar.dma_start(bq_sb[:], bqr[:])
            bk_sb = pp.tile([P, D], bf16, name="bk_sb")
            nc.scalar.dma_start(bk_sb[:], bkr[:])
            bv_sb = pp.tile([P, KC], f32, name="bv_sb")
            nc.scalar.dma_start(bv_sb[:], bvc[:])
            ones_b = pp.tile([P, 1], bf16, name="ones_b")
            nc.any.memset(ones_b[:], 1.0)
            ones_f = pp.tile([1, P], f32, name="ones_f")
            nc.any.memset(ones_f[:], 1.0)

            # ---- Q projection (full) ----
            # First 7 tiles run kc-major across 7 concurrent PSUM banks so the
            # PE consumes each (xt,wq) chunk pair as soon as it lands instead
            # of serializing tile-major behind the full input stream.
            q_sb = pp.tile([P, NCH, D], bf16, name="q_sb")
            k_sb = pp.tile([P, NCH, D], bf16, name="k_sb")
            g0 = [(0, 0), (1, 0), (2, 0), (3, 0), (0, 1), (1, 1), (2, 1)]
            ps_g0 = [psA.tile([P, F], f32, tag="psA", name="ps_q0")
                     for _ in g0]
            for kc in range(KC):
                for t, (nch, jc) in enumerate(g0):
                    nc.tensor.matmul(
                        ps_g0[t][:],
                        xts(kc, nch * P, (nch + 1) * P),
                        wqs(kc, jc * F, (jc + 1) * F),
                        start=(kc == 0), stop=(kc == KC - 1))
            for t, (nch, jc) in enumerate(g0):
                nc.vector.tensor_tensor(
                    q_sb[:, nch, jc * F:(jc + 1) * F],
                    ps_g0[t][:], bq_sb[:, jc * F:(jc + 1) * F], add)
            for nch in range(NCH):
                for jc in range(JC):
                    if (nch, jc) in g0:
                        continue
                    ps = psA.tile([P, F], f32, tag="psA", name="ps_q")
                    for kc in range(KC):
                        nc.tensor.matmul(
                            ps[:],
                            xts(kc, nch * P, (nch + 1) * P),
                            wqs(kc, jc * F, (jc + 1) * F),
                            start=(kc == 0), stop=(kc == KC - 1))
                    nc.vector.tensor_tensor(
                        q_sb[:, nch, jc * F:(jc + 1) * F],
                        ps[:], bq_sb[:, jc * F:(jc + 1) * F], add)

            # scores split into two column blocks (384 / 640); each block is
            # projected (K), contracted (S), and all-reduced independently.
            # The FIRST block is the small one so AR#1 triggers ~12us earlier;
            # the chained AR#2 then also completes earlier, and the larger
            # final out block provides more tail work to hide it.
            s_bounce = [dp.tile([D, w], f16, name=f"s_bounce{h}")
                        for h, (c0, w) in enumerate(SPLITS)]
            s_red = [dp.tile([D, w], f16, name=f"s_red{h}",
                             addr_space="Shared")
                     for h, (c0, w) in enumerate(SPLITS)]
            e_sb = pp.tile([P, KC, D], bf16, name="e_sb")
            for h, (c0, w) in enumerate(SPLITS):
                # K columns for this block
                for nch in range(NCH):
                    for js, jw in jtiles(c0, w):
                        ps = psA.tile([P, jw], f32, tag="psA", name="ps_k")
                        for kc in range(KC):
                            nc.tensor.matmul(
                                ps[:],
                                xts(kc, nch * P, (nch + 1) * P),
                                wk[:, kc, js:js + jw],
                                start=(kc == 0), stop=(kc == KC - 1))
                        nc.vector.tensor_tensor(
                            k_sb[:, nch, js:js + jw],
                            ps[:], bk_sb[:, js:js + jw], add)
                # partial scores for this block: [all qi, these j-cols]
                for qch in range(KC):
                    for js, jw in jtiles(c0, w):
                        st = sp.tile([P, jw], f16, tag="sstage", name="st")
                        ps = psA.tile([P, jw], f32, tag="psA", name="ps_s")
                        for nch in range(NCH):
                            nc.tensor.matmul(
                                ps[:],
                                q_sb[:, nch, qch * P:(qch + 1) * P],
                                k_sb[:, nch, js:js + jw],
                                start=(nch == 0), stop=(nch == NCH - 1))
                        nc.scalar.activation(
                            st[:], ps[:],
                            mybir.ActivationFunctionType.Copy)
                        nc.sync.dma_start(
                            s_bounce[h][qch * P:(qch + 1) * P,
                                        js - c0:js - c0 + jw], st[:])
                nc.gpsimd.collective_compute(
                    "AllReduce", add,
                    replica_groups=[list(range(NCORES))],
                    ins=[s_bounce[h].opt()], outs=[s_red[h].opt()])

            # softmax-input loads + exp on the scalar ring/engine, in ~256KB
            # pieces (small loads serialize ~2us receipts, one big load
            # delays the first exp). Emitted AFTER both AR triggers: exp h0
            # blocks the scalar queue on AR#1, so nothing that gates AR#2
            # may be queued behind it.
            for h, (c0, w) in enumerate(SPLITS):
                s_t = sip.tile([P, KC, w], f16, tag="sin", name="s_t")
                s_rr = s_red[h].rearrange("(ich p) f -> p ich f", p=P)
                for q4 in range(4):
                    nc.scalar.dma_start(
                        s_t[:, 2 * q4:2 * q4 + 2], s_rr[:, 2 * q4:2 * q4 + 2])
                for ich in range(KC):
                    nc.scalar.activation(
                        e_sb[:, ich, c0:c0 + w], s_t[:, ich],
                        mybir.ActivationFunctionType.Exp, scale=NORM)

            # ---- V.T projection (independent of both AllReduces), split in
            # two column halves with the h0 softmax denominator in between so
            # the PE reaches it only after AR#1 + exp h0 are long done ----
            vt_sb = pp.tile([P, KC, NPC], bf16, name="vt_sb")
            den_sb = pp.tile([1, D], f32, name="den_sb")
            rden_sb = pp.tile([P, D], f32, name="rden_sb")

            def vproj_half(jc2):
                for ich in range(KC):
                    ps = psA.tile([P, F], f32, tag="psA", name="ps_v")
                    for kc in range(KC):
                        nc.tensor.matmul(
                            ps[:],
                            wv[:, kc, ich * P:(ich + 1) * P],
                            xts(kc, jc2 * F, (jc2 + 1) * F),
                            start=(kc == 0), stop=(kc == KC - 1))
                    nc.vector.tensor_scalar(
                        vt_sb[:, ich, jc2 * F:(jc2 + 1) * F],
                        ps[:], bv_sb[:, ich:ich + 1], None, add)

            def out_half(h):
                # denominator matmuls are interleaved with the first out
                # tile's accumulation: both consume exp chunk ich as the
                # scalar engine produces it, so nothing serializes behind the
                # full exp stream after the AllReduce lands
                c0, w = SPLITS[h]
                for js, jw in jtiles(c0, w):
                    psd = psB.tile([1, jw], f32, tag="psB", name="psd")
                    for nch in range(NCH):
                        ot = sp.tile([P, jw], bf16, tag="ostage", name="ot")
                        ps = psA.tile([P, jw], f32, tag="psA", name="ps_o")
                        for ich in range(KC):
                            nc.tensor.matmul(
                                ps[:],
                                vt_sb[:, ich, nch * P:(nch + 1) * P],
                                e_sb[:, ich, js:js + jw],
                                start=(ich == 0), stop=(ich == KC - 1))
                            if nch == 0:
                                nc.tensor.matmul(
                                    psd[:], ones_b[:, 0:1],
                                    e_sb[:, ich, js:js + jw],
                                    start=(ich == 0), stop=(ich == KC - 1))
                        if nch == 0:
                            nc.vector.tensor_copy(
                                den_sb[:, js:js + jw], psd[:])
                            psr = psB.tile([P, jw], f32, tag="psB", name="psr")
                            nc.tensor.matmul(
                                psr[:], ones_f[:, 0:P],
                                den_sb[:, js:js + jw],
                                start=True, stop=True)
                            nc.vector.reciprocal(
                                rden_sb[:, js:js + jw], psr[:])
                        nc.vector.tensor_tensor(
                            ot[:], ps[:], rden_sb[:, js:js + jw], mult)
                        nc.sync.dma_start(
                            out[nch * P:(nch + 1) * P, js:js + jw], ot[:])

            vproj_half(0)
            vproj_half(1)
            out_half(0)
            out_half(1)

    nc.compile()
    return nc


def _prep_inputs(x, Wq, bq, Wk, bk, Wv, bv):
    bf16 = ml_dtypes.bfloat16
    xT_all = np.ascontiguousarray(np.asarray(x).astype(bf16).T)
    WqT = np.ascontiguousarray(np.asarray(Wq).astype(bf16).T)
    WkT = np.ascontiguousarray(np.asarray(Wk).astype(bf16).T)
    WvT = np.ascontiguousarray(np.asarray(Wv).astype(bf16).T)
    bqr = np.ascontiguousarray(
        np.broadcast_to(np.asarray(bq, np.float32).astype(bf16), (P, D)))
    bkr = np.ascontiguousarray(
        np.broadcast_to(np.asarray(bk, np.float32).astype(bf16), (P, D)))
    bvc = np.ascontiguousarray(
        np.asarray(bv, np.float32).reshape(KC, P).T)
    in_maps = []
    for c in range(NCORES):
        shard = xT_all[:, c * NPC:(c + 1) * NPC]
        xq = np.ascontiguousarray(np.concatenate(
            [shard[:, 0:F], WqT[:, 0:F], shard[:, F:], WqT[:, F:]], axis=1))
        in_maps.append({
            "XQ": xq, "WkT": WkT, "WvT": WvT,
            "bqr": bqr, "bkr": bkr, "bvc": bvc,
        })
    return in_maps


def _ensure_axon_hooks_stub():
    # bass_utils imports antenv.axon_hooks when tracing is requested (also
    # via the BASS_TRACE env var); this image ships antenv without that
    # submodule, so install a no-op stub to degrade gracefully.
    import sys
    import types
    try:
        import antenv.axon_hooks  # noqa: F401
        return
    except ImportError:
        pass
    mod = types.ModuleType("antenv.axon_hooks")
    mod._hook = None
    mod.set_axon_ntff_profile_hook = lambda h: setattr(mod, "_hook", h)
    mod.get_axon_ntff_profile_hook = lambda: mod._hook
    sys.modules["antenv.axon_hooks"] = mod
    try:
        import antenv
        antenv.axon_hooks = mod
    except ImportError:
        pass


def kernel(x, Wq, bq, Wk, bk, Wv, bv, _trace=False):
    from concourse import bass_utils

    _ensure_axon_hooks_stub()

    if "nc" not in _cache:
        _cache["nc"] = _build_nc()
    nc = _cache["nc"]

    in_maps = _prep_inputs(x, Wq, bq, Wk, bk, Wv, bv)
    res = bass_utils.run_bass_kernel_spmd(
        nc, in_maps, core_ids=list(range(NCORES)), trace=_trace)
    _cache["last_result"] = res
    return np.concatenate(
        [np.asarray(res.results[c]["out"], dtype=np.float32)
         for c in range(NCORES)], axis=0)
